# revision 22
# baseline (speedup 1.0000x reference)
"""Performer (FAVOR+) linear attention on 8 TRN2 NeuronCores.

Sharding: core c handles batch b=c//4 and head group g=c%4 (4 of 16 heads).
Each core computes q/k/v projections for its heads from its batch's x,
runs the per-head softmax-kernel + linear-attention chain, and produces a
partial output projection y_c = o_heads @ Wo_slice.T.  Host sums the 4
partials per batch and adds the bias.

Matmuls use float32r (fp32 with 12 low mantissa bits dropped; 4x faster on
the PE at free-dim>=256).  End-to-end absmax error vs the fp32 reference is
~8e-4 relative to output scale (measured via numpy simulation of the
rounding).
"""
import sys
sys.path.insert(0, '/opt/trn_rl_repo')

import numpy as np
import concourse.bass as bass
import concourse.bacc as bacc
import concourse.tile as tile
from concourse import mybir
from concourse.bass_utils import run_bass_kernel_spmd

F32 = mybir.dt.float32
F32R = mybir.dt.float32r
AX = mybir.AxisListType.X
AF = mybir.ActivationFunctionType

B, N, D = 2, 4096, 1024
H, DH, M = 16, 64, 266            # heads, dim_head, nb_features
HPC = 4                           # heads per core
EPS = 1e-4
CNORM = DH ** -0.25               # data normalizer
RATIO = M ** -0.5
LNR = float(np.log(RATIO))
NT = N // 128                     # 32 n-tiles
# m-chunks over the 267-wide (ones col at 0, then 266 m cols) kp/qp tiles
MCH = [(0, 128), (128, 128), (256, 11)]  # (off, width)
LIMIT = "all"  # debug: p1 | k1 | k2 | q | all


def build():
    nc = bacc.Bacc("TRN2", target_bir_lowering=False, debug=False)

    xT = nc.dram_tensor("xT", [D, N], F32, kind="ExternalInput")
    wqT = nc.dram_tensor("wqT", [D, 256], F32, kind="ExternalInput")
    wkT = nc.dram_tensor("wkT", [D, 256], F32, kind="ExternalInput")
    wvT = nc.dram_tensor("wvT", [D, 256], F32, kind="ExternalInput")
    woP = nc.dram_tensor("woP", [128, 2048], F32, kind="ExternalInput")
    projc = nc.dram_tensor("projc", [DH, M], F32, kind="ExternalInput")
    ident = nc.dram_tensor("ident", [128, 128], F32, kind="ExternalInput")
    y = nc.dram_tensor("y", [N, D], F32, kind="ExternalOutput")
    qkv_d = nc.dram_tensor("qkv_scr", [3, HPC, N, DH], F32, kind="Internal")

    with tile.TileContext(nc) as tc:
        ctx_mgr = tc.tile_pool(name="const", bufs=1)
        with ctx_mgr as cpool, \
             tc.tile_pool(name="stage", bufs=1) as stg, \
             tc.tile_pool(name="stream", bufs=3) as strm, \
             tc.tile_pool(name="big", bufs=2) as big, \
             tc.tile_pool(name="small", bufs=4) as sml, \
             tc.tile_pool(name="ot", bufs=1) as otp_pool, \
             tc.tile_pool(name="ps", bufs=2, space="PSUM") as ps, \
             tc.tile_pool(name="psc", bufs=1, space="PSUM") as psc, \
             tc.tile_pool(name="pst", bufs=1, space="PSUM") as pst:

            # ---- constants / weights ----
            wq_r = cpool.tile([128, 8, 256], F32R, tag="wq")
            wk_r = cpool.tile([128, 8, 256], F32R, tag="wk")
            wv_r = cpool.tile([128, 8, 256], F32R, tag="wv")
            wo_r = cpool.tile([128, 2048], F32R, tag="wo")
            projc_r = cpool.tile([DH, M], F32R, tag="pj")
            ident_f = cpool.tile([128, 128], F32, tag="idf")
            ident_r = cpool.tile([128, 128], F32R, tag="idr")
            ones1_f = cpool.tile([1, 128], F32, tag="o1f")
            ones1_r = cpool.tile([1, 128], F32R, tag="o1r")
            ones128 = cpool.tile([128, 1], F32R, tag="o128")

            for dst, src in ((wq_r, wqT), (wk_r, wkT), (wv_r, wvT)):
                st = stg.tile([128, 8, 256], F32, tag="wstage")
                nc.sync.dma_start(st[:], src.ap().rearrange("(c p) n -> p c n", p=128))
                nc.vector.tensor_copy(dst[:], st[:])
            st = stg.tile([128, 2048], F32, tag="wstage")
            nc.sync.dma_start(st[:], woP.ap())
            nc.vector.tensor_copy(wo_r[:], st[:])
            st = stg.tile([DH, M], F32, tag="pstage")
            nc.sync.dma_start(st[:], projc.ap())
            nc.vector.tensor_copy(projc_r[:], st[:])
            nc.sync.dma_start(ident_f[:], ident.ap())
            nc.scalar.copy(ident_r[:], ident_f[:])
            nc.vector.memset(ones1_f[:], 1.0)
            nc.scalar.copy(ones1_r[:], ones1_f[:])
            nc.scalar.activation(ones128[:], ident_f[:, 0:1], AF.Identity,
                                 bias=1.0, scale=0.0)

            # ---- phase 1: QKV projections, spilled to DRAM scratch ----
            for j in range(NT):
                xt = strm.tile([128, 8, 128], F32, tag="xt", bufs=2)
                nc.sync.dma_start(
                    xt[:], xT.ap().rearrange("(c p) n -> p c n", p=128)[:, :, j*128:(j+1)*128])
                xt_r = strm.tile([128, 8, 128], F32R, tag="xtr", bufs=2)
                nc.vector.tensor_copy(xt_r[:], xt[:])
                for ti, w_r in ((0, wq_r), (1, wk_r), (2, wv_r)):
                    acc = ps.tile([128, 256], F32, tag="b256")
                    for dchunk in range(8):
                        nc.tensor.matmul(acc[:], xt_r[:, dchunk, :], w_r[:, dchunk, :],
                                         start=(dchunk == 0), stop=(dchunk == 7))
                    sb = strm.tile([128, 4, DH], F32, tag=f"qkv{ti}", bufs=2)
                    nc.vector.tensor_copy(sb[:], acc[:].rearrange("p (h e) -> p h e", e=DH))
                    nc.sync.dma_start(
                        qkv_d.ap()[ti].rearrange("h (j p) e -> j p h e", p=128)[j], sb[:])

            # ---- per-head chain ----
            nheads = 0 if LIMIT == "p1" else (1 if LIMIT in ("k1", "k2", "q") else HPC)  # "heads": 4 heads, no P3
            for h in range(nheads):
                kb = big.tile([128, NT, DH], F32, tag="hb", bufs=3, name="kb")
                nc.sync.dma_start(
                    kb[:], qkv_d.ap()[1, h].rearrange("(j p) e -> p j e", p=128))
                dashk = big.tile([128, NT, M], F32, tag="dashk", bufs=1)
                rmaxb = sml.tile([128, NT], F32, tag="rmaxb", bufs=2)
                diagk = sml.tile([128, NT], F32, tag="diagk", bufs=2)

                # K1: dash_k tiles, running stats
                for j in range(NT):
                    sqj = strm.tile([128, DH], F32, tag="sqj")
                    nc.scalar.activation(sqj[:], kb[:, j, :], AF.Square,
                                         accum_out=diagk[:, j:j+1])
                    ktp = pst.tile([DH, 128], F32, tag="tp64")
                    nc.tensor.transpose(ktp[:], kb[:, j, :], ident_f[:])
                    kt = strm.tile([DH, 128], F32R, tag="kt")
                    nc.scalar.copy(kt[:], ktp[:])
                    dash = ps.tile([128, M], F32, tag="b256")
                    nc.tensor.matmul(dash[:], kt[:], projc_r[:], start=True, stop=True)
                    nc.vector.tensor_copy(dashk[:, j, :], dash[:])
                    if j % 4 == 3:
                        nc.vector.reduce_max(rmaxb[:, j-3:j+1],
                                             dashk[:, j-3:j+1, :], axis=AX)

                if LIMIT == "k1":
                    continue
                # global max -> per-partition broadcast mkb
                gmax = sml.tile([128, 1], F32, tag="gmax")
                nc.vector.reduce_max(gmax[:], rmaxb[:], axis=AX)
                gm_ps = pst.tile([1, 128], F32, tag="tp64")
                nc.tensor.transpose(gm_ps[:], gmax[:], ident_f[:])
                gmrow = sml.tile([1, 128], F32, tag="gmrow")
                nc.vector.tensor_copy(gmrow[:], gm_ps[:])
                mk = sml.tile([1, 1], F32, tag="mk")
                nc.vector.reduce_max(mk[:], gmrow[:], axis=AX)
                mk_ps = pst.tile([128, 1], F32, tag="tp64")
                nc.tensor.matmul(mk_ps[:], ones1_f[:], mk[:], start=True, stop=True)
                mkl = sml.tile([128, 1], F32, tag="mkl")
                # mkl = lnr - mk
                nc.vector.tensor_scalar(mkl[:], mk_ps[:], -1.0, LNR,
                                        op0=mybir.AluOpType.mult, op1=mybir.AluOpType.add)

                # batched per-tile exp bias: -0.5c^2*diag + (lnr - mk)
                biaskb = sml.tile([128, NT], F32, tag="biaskb", bufs=2)
                nc.vector.tensor_scalar(biaskb[:], diagk[:],
                                        -0.5 * CNORM * CNORM, mkl[:],
                                        op0=mybir.AluOpType.mult,
                                        op1=mybir.AluOpType.add)

                # K2: kp = exp(dash - 0.5c^2*diag - mk + lnr), context accumulation
                vb = big.tile([128, NT, DH], F32, tag="hb", bufs=3, name="vb")
                nc.sync.dma_start(
                    vb[:], qkv_d.ap()[2, h].rearrange("(j p) e -> p j e", p=128))
                vxb = big.tile([128, NT, 66], F32R, tag="vxb", bufs=2)
                nc.scalar.copy(vxb[:, :, 0:DH], vb[:])
                nc.scalar.activation(vxb[:, :, DH:66], vb[:, :, 0:2], AF.Identity,
                                     bias=1.0, scale=0.0)
                ctx_ps = [psc.tile([128, 66], F32, tag=f"ctx{mc}", name=f"ctxp{mc}") for mc in range(2)]
                ctx_ps.append(psc.tile([11, 66], F32, tag="ctx2", name="ctxp2"))
                colsum_ps = pst.tile([1, 66], F32, tag="oe", name="colsum_ps")
                for j in range(NT):
                    # kp col 0 is a ones column (for k_cumsum); cols 1..266 = kp
                    kp = strm.tile([128, M + 1], F32R, tag="kp")
                    nc.scalar.activation(kp[:, 1:M+1], dashk[:, j, :], AF.Exp,
                                         bias=biaskb[:, j:j+1], scale=1.0)
                    nc.scalar.activation(kp[:, 0:1], kp[:, 1:2], AF.Identity,
                                         bias=1.0, scale=0.0)
                    for mc, (off, w) in enumerate(MCH):
                        nc.tensor.matmul(ctx_ps[mc][:], kp[:, off:off+w],
                                         vxb[:, j, :], start=(j == 0), stop=False)
                    # colsum_v accumulator (own group so it can be read while
                    # the ctx groups are still open)
                    nc.tensor.matmul(colsum_ps[:], kp[:, 0:1], vxb[:, j, :],
                                     start=(j == 0), stop=(j == NT - 1))

                # eps-correction: ctx += ratio*eps * ones x colsum_v.  Row 0 of
                # chunk0 (the kp-ones-column product) gets corrupted by the
                # correction; it is overwritten with the qp-side eps row below.
                colsum = sml.tile([1, 66], F32R, tag="colsum")
                nc.scalar.mul(colsum[:], colsum_ps[:], RATIO * EPS)
                for mc, (off, w) in enumerate(MCH):
                    nc.tensor.matmul(ctx_ps[mc][:], ones1_r[:, 0:w],
                                     colsum[:], start=False, stop=True)
                ctx_s = [big.tile([128, 66], F32R, tag=f"ctxs{mc}", name=f"ctxs{mc}") for mc in range(2)]
                ctx_s.append(big.tile([11, 66], F32R, tag="ctxs2", name="ctxs2"))
                for mc in range(3):
                    nc.scalar.copy(ctx_s[mc][:], ctx_ps[mc][:])
                # sum of corrected ctx over all rows, minus the (corrected)
                # row 0, = sum over real m rows
                smc_ps = pst.tile([1, 66], F32, tag="tp64")
                for mc, (off, w) in enumerate(MCH):
                    nc.tensor.matmul(smc_ps[:], ones128[0:w, :], ctx_s[mc][:],
                                     start=(mc == 0), stop=(mc == 2))
                smc_s = sml.tile([1, 66], F32, tag="smcs")
                nc.vector.tensor_copy(smc_s[:], smc_ps[:])
                eps_t = sml.tile([1, 66], F32, tag="epst")
                nc.vector.tensor_sub(eps_t[:], smc_s[:], ctx_s[0][0:1, :].bitcast(F32))
                # ctx chunk0 row 0 := ratio*eps*sumctx (consumed by qp ones col)
                nc.scalar.mul(ctx_s[0][0:1, :], eps_t[:], RATIO * EPS)

                if LIMIT == "k2":
                    continue
                # Q pass
                qb = big.tile([128, NT, DH], F32, tag="hb", bufs=3, name="qb")
                nc.sync.dma_start(
                    qb[:], qkv_d.ap()[0, h].rearrange("(j p) e -> p j e", p=128))
                for j in range(NT):
                    diagq = sml.tile([128, 1], F32, tag="diagq")
                    sqj = strm.tile([128, DH], F32, tag="sqj")
                    nc.scalar.activation(sqj[:], qb[:, j, :], AF.Square,
                                         accum_out=diagq[:])
                    qtp = pst.tile([DH, 128], F32, tag="tp64")
                    nc.tensor.transpose(qtp[:], qb[:, j, :], ident_f[:])
                    qt = strm.tile([DH, 128], F32R, tag="kt")
                    nc.scalar.copy(qt[:], qtp[:])
                    dash = ps.tile([128, M], F32, tag="b256")
                    nc.tensor.matmul(dash[:], qt[:], projc_r[:], start=True, stop=True)
                    rmaxq = sml.tile([128, 1], F32, tag="rmaxq")
                    nc.vector.reduce_max(rmaxq[:], dash[:], axis=AX)
                    biasq = sml.tile([128, 1], F32, tag="biasq")
                    nc.vector.tensor_scalar(biasq[:], diagq[:],
                                            -0.5 * CNORM * CNORM, LNR,
                                            op0=mybir.AluOpType.mult,
                                            op1=mybir.AluOpType.add)
                    nc.vector.tensor_sub(biasq[:], biasq[:], rmaxq[:])
                    qp = strm.tile([128, M + 1], F32R, tag="qp")
                    nc.scalar.activation(qp[:, 1:M+1], dash[:], AF.Exp,
                                         bias=biasq[:], scale=1.0)
                    nc.scalar.activation(qp[:, 0:1], qp[:, 1:2], AF.Identity,
                                         bias=1.0, scale=0.0)
                    qpt_ps = ps.tile([128, 384], F32R, tag="qpt", bufs=1)
                    for mc, (off, w) in enumerate(MCH):
                        nc.tensor.transpose(qpt_ps[0:w, mc*128:mc*128+128],
                                            qp[:, off:off+w], ident_r[:])
                    qpt = strm.tile([128, 384], F32R, tag="qpts")
                    nc.vector.tensor_copy(qpt[:, 0:256], qpt_ps[:, 0:256])
                    nc.vector.tensor_copy(qpt[0:11, 256:384], qpt_ps[0:11, 256:384])
                    oe_ps = pst.tile([128, 66], F32, tag="oe")
                    for mc, (off, w) in enumerate(MCH):
                        nc.tensor.matmul(oe_ps[:], qpt[0:w, mc*128:mc*128+128],
                                         ctx_s[mc][:], start=(mc == 0), stop=(mc == 2))
                    oe = strm.tile([128, 66], F32, tag="oes")
                    nc.scalar.copy(oe[:], oe_ps[:])
                    dinv = sml.tile([128, 1], F32, tag="dinv")
                    nc.vector.reciprocal(dinv[:], oe[:, DH:DH+1])
                    osc = strm.tile([128, DH], F32R, tag="osc")
                    nc.vector.tensor_scalar_mul(osc[:], oe[:, 0:DH], dinv[:])
                    ot_ps = pst.tile([DH, 128], F32R, tag="tp64")
                    nc.tensor.transpose(ot_ps[:], osc[:], ident_r[:])
                    if h == 0 and j == 0:
                        otb = otp_pool.tile([128, 2, N], F32R, tag="otb")
                    pb = (h % 2) * 64
                    nc.scalar.copy(otb[pb:pb+DH, h // 2, j*128:(j+1)*128], ot_ps[:])

            # ---- phase 3: output projection ----
            if LIMIT != "all":
                for j in range(NT):
                    y_z = strm.tile([128, 1024], F32, tag="ys", bufs=2)
                    nc.vector.memset(y_z[:], 0.0)
                    nc.sync.dma_start(y.ap()[j*128:(j+1)*128, :], y_z[:])
            # Each matmul contracts a head PAIR (K=128): otb chunk ch stacks
            # heads 2ch (rows 0:64) and 2ch+1 (rows 64:128); wo_r stacks the
            # matching Wo rows.  All operands at base partition 0 — mixing
            # base partitions inside one PSUM accumulation group is fatal.
            for j in range(NT if LIMIT == "all" else 0):
                y_ps = [psc.tile([128, 512], F32, tag=f"ctx{nb}", name=f"yps{nb}") for nb in range(2)]
                for nb in range(2):
                    for ch in range(2):
                        nc.tensor.matmul(y_ps[nb][:],
                                         otb[:, ch, j*128:(j+1)*128],
                                         wo_r[:, ch*1024 + nb*512:
                                              ch*1024 + nb*512 + 512],
                                         start=(ch == 0), stop=(ch == 1))
                y_s = strm.tile([128, 1024], F32, tag="ys", bufs=2)
                for nb in range(2):
                    nc.vector.tensor_copy(y_s[:, nb*512:(nb+1)*512], y_ps[nb][:])
                nc.sync.dma_start(y.ap()[j*128:(j+1)*128, :], y_s[:])

    nc.compile()
    return nc


_prog = None


def _build_in_maps(inputs):
    return _make_in_maps(**inputs)


def _make_in_maps(x, Wq, Wk, Wv, Wo, bo, proj):
    x = np.asarray(x, np.float32)
    projc = np.ascontiguousarray(CNORM * np.asarray(proj, np.float32).T)
    identm = np.eye(128, dtype=np.float32)
    xTb = [np.ascontiguousarray(x[b].T) for b in range(B)]
    in_maps = []
    for c in range(8):
        b, g = c // 4, c % 4
        hs, he = g * 256, g * 256 + 256
        woT = np.asarray(Wo, np.float32)[:, hs:he].T          # [256, 1024]
        woP = np.concatenate([woT[:128], woT[128:]], axis=1)  # [128, 2048]
        in_maps.append({
            "xT": xTb[b],
            "wqT": np.ascontiguousarray(np.asarray(Wq, np.float32)[hs:he].T),
            "wkT": np.ascontiguousarray(np.asarray(Wk, np.float32)[hs:he].T),
            "wvT": np.ascontiguousarray(np.asarray(Wv, np.float32)[hs:he].T),
            "woP": np.ascontiguousarray(woP),
            "projc": projc,
            "ident": identm,
        })
    return in_maps


def kernel(x, Wq, Wk, Wv, Wo, bo, proj):
    global _prog
    if _prog is None:
        _prog = build()
    in_maps = _make_in_maps(x, Wq, Wk, Wv, Wo, bo, proj)
    res = run_bass_kernel_spmd(_prog, in_maps, core_ids=list(range(8)))
    out = np.zeros((B, N, D), np.float32)
    for c in range(8):
        out[c // 4] += res.results[c]["y"]
    out += np.asarray(bo, np.float32)[None, None, :]
    return out



# revision 23
# speedup vs baseline: 1.1961x; 1.1961x over previous
"""Performer (FAVOR+) linear attention on 8 TRN2 NeuronCores — v3.

Sharding: core c handles batch b=c//4 and head group g=c%4 (4 of 16 heads).

Design highlights:
- q/k computed directly head-dim-major (lhsT=W, rhs=x): kqaug packs kT in
  partitions 0:64 and qT in 64:128, one [128, 4, N] fp16 tile; v n-major.
  QKV stays resident in SBUF (no DRAM scratch round-trip).
- 16-bit matmul operands; x and proj stay fp32 via zero-copy float32r
  bitcast (moving >= 256 keeps full rate).  End-to-end err ~5e-3.
- diag columns (-0.5 c^2 sum k^2) for all heads/tiles computed in phase 1
  via tiny matmuls against a constant column; squares on the Pool engine.
- K pass stores dash-diag + its rowmax in one fused DVE op
  (tensor_tensor_reduce); the exact reference max is reconstructed as
  max(rowmax(dash-diag) + diag) and reduced via gpsimd partition_all_reduce.
- Q pass needs no pre-exp rowmax: the e^{-rowmax} row factor cancels
  between numerator and denominator; the eps term gets its e^{rowmax}
  weight through qp column 0 = rowmax(exp'd tile) * exp(diag - ln ratio).
- kp/qp layout [ones | dead-zero | 266 features] keeps the feature block
  4-byte aligned for DVE 16-bit mode.
- Head-pipelined K phase (K1 of head h interleaved with K2 of h-1, two
  dashk buffers) and a unified Q loop (heads round-robin per n-slice) with
  the output projection + DMA interleaved so the store never tails.
- eps regularization via the same rank-one corrections as the reference
  requires (colsum trick + ctx row-0 rewrite).
"""
import sys
sys.path.insert(0, '/opt/trn_rl_repo')

import numpy as np
import concourse.bass as bass
import concourse.bacc as bacc
import concourse.tile as tile
from concourse import mybir
from concourse.bass_utils import run_bass_kernel_spmd

F32 = mybir.dt.float32
F32R = mybir.dt.float32r
BF16 = mybir.dt.bfloat16
FP16 = mybir.dt.float16
AX = mybir.AxisListType.X
AF = mybir.ActivationFunctionType

B, N, D = 2, 4096, 1024
H, DH, M = 16, 64, 266            # heads, dim_head, nb_features
HPC = 4                           # heads per core
EPS = 1e-4
CNORM = DH ** -0.25               # data normalizer
RATIO = M ** -0.5
LNR = float(np.log(RATIO))
NT = N // 128                     # 32 n-tiles
NT2 = N // 256                    # 16 phase-1 tiles
# kp/qp column layout: col0 = ones/eps-weight, col1 = dead zero (keeps the
# exp'd m block 4B-aligned for DVE 2x), cols 2:268 = the 266 m features.
MCH = [(0, 128), (128, 128), (256, 12)]  # chunks over width 268
MW = 268


def build():
    nc = bacc.Bacc("TRN2", target_bir_lowering=False, debug=False)

    xT = nc.dram_tensor("xT", [D, N], F32, kind="ExternalInput")
    wqT = nc.dram_tensor("wqT", [D, 256], F32, kind="ExternalInput")
    wkT = nc.dram_tensor("wkT", [D, 256], F32, kind="ExternalInput")
    wvT = nc.dram_tensor("wvT", [D, 256], F32, kind="ExternalInput")
    woP = nc.dram_tensor("woP", [128, 2048], F32, kind="ExternalInput")
    projc = nc.dram_tensor("projc", [DH, M], F32, kind="ExternalInput")
    ident = nc.dram_tensor("ident", [128, 128], F32, kind="ExternalInput")
    y = nc.dram_tensor("y", [N, D], F32, kind="ExternalOutput")

    with tile.TileContext(nc) as tc:
        with tc.tile_pool(name="const", bufs=1) as cpool, \
             tc.tile_pool(name="stage", bufs=2) as stg, \
             tc.tile_pool(name="strm", bufs=3) as strm, \
             tc.tile_pool(name="sml", bufs=4) as sml:

            # ---- persistent SBUF stores ----
            kqaug = cpool.tile([128, HPC, N], FP16, tag="kqaug")  # kT | qT halves
            vaug = cpool.tile([128, NT, HPC, 66], BF16, tag="vaug")
            dashk_t = [cpool.tile([128, NT, M], F32, tag=f"dashk{i}",
                                  name=f"dashk{i}") for i in range(2)]
            dgall_k = cpool.tile([128, HPC, NT], F32, tag="dgk")  # -diag (k)
            dgall_q = cpool.tile([128, HPC, NT], F32, tag="dgq")  # -diag (q)
            wq_h = cpool.tile([128, 8, 256], FP16, tag="wqh")
            wk_h = cpool.tile([128, 8, 256], FP16, tag="wkh")
            wv_h = cpool.tile([128, 8, 256], FP16, tag="wvh")
            wo_b = cpool.tile([128, 2048], BF16, tag="wob")
            projc2 = cpool.tile([128, M], FP16, tag="pj")         # both halves
            projst = cpool.tile([128, M], F32, tag="pjst")
            ident_b = cpool.tile([128, 128], BF16, tag="idb")
            halfvec = cpool.tile([128, 1], FP16, tag="hvec")      # -0.5c^2
            ones1_b = cpool.tile([1, 128], BF16, tag="o1b")
            ones128 = cpool.tile([128, 1], BF16, tag="o128")

            # ---- load constants ----
            for dst, srcw in ((wq_h, wqT), (wk_h, wkT), (wv_h, wvT)):
                st = stg.tile([128, 8, 256], F32, tag="xst")
                nc.sync.dma_start(st[:], srcw.ap().rearrange("(c p) n -> p c n", p=128))
                nc.vector.tensor_copy(dst[:], st[:])
            st = stg.tile([128, 8, 256], F32, tag="xst")
            nc.sync.dma_start(st[:].rearrange("p c n -> p (c n)"), woP.ap())
            nc.vector.tensor_copy(wo_b[:], st[:].rearrange("p c n -> p (c n)"))
            nc.sync.dma_start(projst[0:DH, :], projc.ap())
            nc.sync.dma_start(projst[DH:128, :], projc.ap())
            nc.vector.tensor_copy(projc2[:], projst[:])
            st = stg.tile([128, 8, 256], F32, tag="xst")
            nc.sync.dma_start(st[:, 0, 0:128], ident.ap())
            nc.vector.tensor_copy(ident_b[:], st[:, 0, 0:128])
            nc.gpsimd.memset(halfvec[:], -0.5 * CNORM * CNORM)
            nc.gpsimd.memset(ones1_b[:], 1.0)
            nc.gpsimd.memset(ones128[:], 1.0)
            nc.gpsimd.memset(vaug[:, :, :, DH:66], 1.0)


            # ---- phase 1: QKV into SBUF + all diag columns ----
            with tc.tile_pool(name="ps1", bufs=2, space="PSUM") as ps1, \
                 tc.tile_pool(name="psg", bufs=1, space="PSUM") as psg:
                dgk_ps = psg.tile([128, 128], F32, tag="dgkp")
                dgq_ps = psg.tile([128, 128], F32, tag="dgqp")
                for j2 in range(NT2):
                    xst = stg.tile([128, 8, 256], F32, tag="xst")
                    nc.sync.dma_start(
                        xst[:], xT.ap().rearrange("(c p) n -> p c n", p=128)
                        [:, :, j2 * 256:(j2 + 1) * 256])
                    xh = strm.tile([128, 8, 256], FP16, tag="xh", bufs=2)
                    nc.vector.tensor_copy(xh[:, 0:4, :], xst[:, 0:4, :])
                    nc.scalar.copy(xh[:, 4:8, :], xst[:, 4:8, :])
                    xr = xh[:]
                    ns = slice(j2 * 256, (j2 + 1) * 256)
                    for qk, (rows, w_r) in enumerate(
                            ((slice(DH, 128), wq_h), (slice(0, DH), wk_h))):
                        for half in range(2):
                            acc = ps1.tile([128, 256], F32, tag="qk")
                            for dc in range(8):
                                nc.tensor.matmul(
                                    acc[:], w_r[:, dc, half * 128:half * 128 + 128],
                                    xr[:, dc, :], start=(dc == 0), stop=(dc == 7))
                            for hh in range(2):
                                h = half * 2 + hh
                                if (half + hh) % 2 == 0:
                                    nc.vector.tensor_copy(
                                        kqaug[rows, h, ns],
                                        acc[hh * DH:(hh + 1) * DH, :])
                                else:
                                    nc.scalar.copy(
                                        kqaug[rows, h, ns],
                                        acc[hh * DH:(hh + 1) * DH, :])
                    for h in range(HPC):
                        sq = strm.tile([128, 256], FP16, tag="sq", bufs=3)
                        nc.gpsimd.tensor_mul(sq[0:DH, :], kqaug[0:DH, h, ns],
                                             kqaug[0:DH, h, ns])
                        nc.gpsimd.tensor_mul(sq[DH:128, :], kqaug[DH:128, h, ns],
                                             kqaug[DH:128, h, ns])
                        for t in range(2):
                            j = j2 * 2 + t
                            nc.tensor.matmul(
                                dgk_ps[:, h * NT + j:h * NT + j + 1],
                                sq[0:DH, t * 128:(t + 1) * 128],
                                halfvec[0:DH, :], start=True, stop=True)
                            nc.tensor.matmul(
                                dgq_ps[:, h * NT + j:h * NT + j + 1],
                                sq[DH:128, t * 128:(t + 1) * 128],
                                halfvec[DH:128, :], start=True, stop=True)
                    for nh in range(2):
                        j = j2 * 2 + nh
                        acc = ps1.tile([128, 256], F32, tag="v")
                        for dc in range(8):
                            nc.tensor.matmul(
                                acc[:], xr[:, dc, nh * 128:nh * 128 + 128],
                                wv_h[:, dc, :], start=(dc == 0), stop=(dc == 7))
                        nc.vector.tensor_copy(
                            vaug[:, j, :, 0:DH],
                            acc[:].rearrange("p (h e) -> p h e", e=DH))
                nc.vector.tensor_copy(
                    dgall_k[:].rearrange("p h j -> p (h j)"), dgk_ps[:])
                nc.scalar.copy(
                    dgall_q[:].rearrange("p h j -> p (h j)"), dgq_ps[:])

            # ---- K phases (head-pipelined) + unified Q/Y loop ----
            with tc.tile_pool(name="psm", bufs=1, space="PSUM") as psm:
                ctx0_t = psm.tile([128, 512], F32, tag="ctx0")
                ctx1_t = psm.tile([128, 512], F32, tag="ctx1")
                b4_t = psm.tile([128, 512], F32, tag="b4")
                b5_t = psm.tile([128, 512], F32, tag="b5")
                qpt_t = [psm.tile([128, 2, 512], BF16, tag=f"qpt{i}",
                                  name=f"qpt_t{i}") for i in range(2)]
                dps_t = [psm.tile([128, 512], F32, tag=f"dps{i}",
                                  name=f"dps_t{i}") for i in range(2)]

                mkl_h = {}
                ctx_s_h = {}
                colsum_h = {}

                def emit_k1_tiles(h, j0, j1):
                    dashk = dashk_t[h % 2]
                    for j in range(j0, j1):
                        dk_ps = dps_t[j % 2][:, 0:M]
                        nc.tensor.matmul(
                            dk_ps[:], kqaug[0:DH, h, j * 128:(j + 1) * 128],
                            projc2[0:DH, :], start=True, stop=True)
                        # store dash - diag (bias column via tensor_scalar)
                        eng = nc.vector if j % 2 == 0 else nc.scalar
                        if j % 2 == 0:
                            nc.vector.tensor_scalar(
                                dashk[:, j, :], dk_ps[:], 1.0,
                                dgall_k[:, h, j:j + 1],
                                op0=mybir.AluOpType.mult,
                                op1=mybir.AluOpType.add)
                        else:
                            nc.scalar.activation(
                                dashk[:, j, :], dk_ps[:], AF.Identity,
                                bias=dgall_k[:, h, j:j + 1], scale=1.0)
                        if j % 4 == 3:
                            nc.vector.reduce_max(
                                rmaxb_h[h][:, j - 3:j + 1],
                                dashk[:, j - 3:j + 1, :], axis=AX)

                def emit_mk_chain(h):
                    # mk = max over all of rowmax(dash-diag) + diag
                    rpd = sml.tile([128, NT], F32, tag="rpd")
                    nc.vector.tensor_sub(rpd[:], rmaxb_h[h][:], dgall_k[:, h, :])
                    gmax = sml.tile([128, 1], F32, tag="gmax")
                    nc.vector.reduce_max(gmax[:], rpd[:], axis=AX)
                    mkb = sml.tile([128, 1], F32, tag="mkb")
                    nc.gpsimd.partition_all_reduce(
                        mkb[:], gmax[:], channels=128,
                        reduce_op=bass.bass_isa.ReduceOp.max)
                    mkl = sml.tile([128, 1], F32, tag="mkl")
                    nc.vector.tensor_scalar(mkl[:], mkb[:], -1.0, LNR,
                                            op0=mybir.AluOpType.mult,
                                            op1=mybir.AluOpType.add)
                    mkl_h[h] = mkl

                def emit_k2_group(h, jj):
                    dashk = dashk_t[h % 2]
                    ctx_ps = [ctx0_t[:, 0:66], ctx1_t[:, 0:66], b4_t[0:12, 0:66]]
                    colsum_ps = b5_t[0:1, 0:66]
                    kp = strm.tile([128, 4, MW], BF16, tag="kp", bufs=2)
                    nc.scalar.activation(
                        kp[:, :, 2:MW], dashk[:, jj * 4:(jj + 1) * 4, :],
                        AF.Exp, bias=mkl_h[h][:], scale=1.0)
                    nc.gpsimd.memset(kp[:, :, 0:1], 1.0)
                    nc.gpsimd.memset(kp[:, :, 1:2], 0.0)
                    for t in range(4):
                        j = jj * 4 + t
                        for mc, (off, w) in enumerate(MCH):
                            nc.tensor.matmul(
                                ctx_ps[mc], kp[:, t, off:off + w],
                                vaug[:, j, h, :], start=(j == 0), stop=False)
                        nc.tensor.matmul(
                            colsum_ps, kp[:, t, 0:1], vaug[:, j, h, :],
                            start=(j == 0), stop=(j == NT - 1))

                def emit_eps_chain(h):
                    ctx_ps = [ctx0_t[:, 0:66], ctx1_t[:, 0:66], b4_t[0:12, 0:66]]
                    colsum_ps = b5_t[0:1, 0:66]
                    colsum = sml.tile([1, 66], BF16, tag="colsum")
                    nc.scalar.mul(colsum[:], colsum_ps, RATIO * EPS)
                    for mc, (off, w) in enumerate(MCH):
                        nc.tensor.matmul(ctx_ps[mc], ones1_b[:, 0:w],
                                         colsum[:], start=False, stop=True)
                    ctx_s = [sml.tile([128, 66], BF16, tag=f"cs{mc}",
                                      name=f"cs{h}{mc}") for mc in range(2)]
                    ctx_s.append(sml.tile([12, 66], BF16, tag="cs2",
                                          name=f"cs{h}2"))
                    for mc in range(3):
                        nc.scalar.copy(ctx_s[mc][:], ctx_ps[mc])
                    smc_ps = b4_t[0:1, 259:325]
                    for mc, (off, w) in enumerate(MCH):
                        nc.tensor.matmul(smc_ps, ones128[0:w, :], ctx_s[mc][:],
                                         start=(mc == 0), stop=(mc == 2))
                    smc_s = sml.tile([1, 66], F32, tag="smcs")
                    nc.vector.tensor_copy(smc_s[:], smc_ps)
                    eps_t = sml.tile([1, 66], F32, tag="epst")
                    nc.vector.tensor_sub(eps_t[:], smc_s[:], ctx_s[0][0:1, :])
                    nc.vector.tensor_sub(eps_t[:], eps_t[:], colsum[:])
                    nc.scalar.mul(ctx_s[0][0:1, :], eps_t[:], RATIO * EPS)
                    ctx_s_h[h] = ctx_s

                rmaxb_h = {h: sml.tile([128, NT], F32, tag="rmaxb",
                                       name=f"rmaxb{h}") for h in range(HPC)}

                # pipeline: K1(h) runs against K2(h-1)
                emit_k1_tiles(0, 0, NT)
                emit_mk_chain(0)
                for h in range(1, HPC):
                    for u in range(NT // 4):
                        emit_k1_tiles(h, u * 4, u * 4 + 4)
                        emit_k2_group(h - 1, u)
                    emit_mk_chain(h)
                    emit_eps_chain(h - 1)
                for u in range(NT // 4):
                    emit_k2_group(HPC - 1, u)
                emit_eps_chain(HPC - 1)

                # per-head Q constants
                bqb_h, edg_h = {}, {}
                for h in range(HPC):
                    bqb = sml.tile([128, NT], F32, tag="bqb", name=f"bqb{h}")
                    # bqb = lnr + dgq (dgq = -diag); no rowmax — the e^{-rq}
                    # row factor cancels in oe/D; the eps term gets its e^{rq}
                    # weight via qp col 0 = rowmax(exp) * edg.
                    nc.vector.tensor_scalar(bqb[:], dgall_q[:, h, :], 1.0, LNR,
                                            op0=mybir.AluOpType.mult,
                                            op1=mybir.AluOpType.add)
                    edg = sml.tile([128, NT], F32, tag="edg", name=f"edg{h}")
                    # edg = exp(diag - lnr) = exp(-bqb)
                    nc.scalar.activation(edg[:], bqb[:], AF.Exp, scale=-1.0)
                    bqb_h[h], edg_h[h] = bqb, edg

                # ---- unified Q + output-projection loop ----
                for jj in range(NT // 2):
                    otr = strm.tile([128, 2, 256], BF16, tag="otr", bufs=2)
                    for h in range(HPC):
                        pp = (jj * HPC + h) % 2
                        qpt_ps = qpt_t[pp][:, :, 0:384]
                        ot_ps = qpt_t[pp][0:DH, :, 384:512]
                        oo = 66 + pp * 132
                        oe_ps = b5_t[:, oo:oo + 132].rearrange(
                            "p (t e) -> p t e", e=66)
                        for t in range(2):
                            j = jj * 2 + t
                            dq_ps = dps_t[(h * 2 + t) % 2][:, 0:M]
                            nc.tensor.matmul(
                                dq_ps[:], kqaug[DH:128, h, j * 128:(j + 1) * 128],
                                projc2[DH:128, :], start=True, stop=True)
                            qp = strm.tile([128, MW], BF16, tag="qp", bufs=4)
                            nc.scalar.activation(qp[:, 2:MW], dq_ps[:], AF.Exp,
                                                 bias=bqb_h[h][:, j:j + 1],
                                                 scale=1.0)
                            nc.gpsimd.memset(qp[:, 1:2], 0.0)
                            rqm = sml.tile([128, 1], F32, tag="rqm")
                            nc.vector.reduce_max(rqm[:], qp[:, 2:MW], axis=AX)
                            nc.vector.tensor_mul(qp[:, 0:1], rqm[:],
                                                 edg_h[h][:, j:j + 1])
                            for mc in (1, 2, 0):  # chunk 0 last (needs col 0)
                                off, w = MCH[mc]
                                nc.tensor.transpose(
                                    qpt_ps[0:w, t, mc * 128:mc * 128 + 128],
                                    qp[:, off:off + w], ident_b[:])
                        qpt = strm.tile([128, 2, 384], BF16, tag="qpts", bufs=2)
                        nc.vector.tensor_copy(qpt[:, 0, 0:256], qpt_ps[:, 0, 0:256])
                        nc.vector.tensor_copy(qpt[0:12, 0, 256:384],
                                              qpt_ps[0:12, 0, 256:384])
                        nc.scalar.copy(qpt[:, 1, 0:256], qpt_ps[:, 1, 0:256])
                        nc.scalar.copy(qpt[0:12, 1, 256:384],
                                       qpt_ps[0:12, 1, 256:384])
                        ctx_s = ctx_s_h[h]
                        for t in range(2):
                            for mc, (off, w) in enumerate(MCH):
                                nc.tensor.matmul(
                                    oe_ps[:, t, :],
                                    qpt[0:w, t, mc * 128:mc * 128 + 128],
                                    ctx_s[mc][:], start=(mc == 0), stop=(mc == 2))
                        osc = strm.tile([128, 2, DH], BF16, tag="osc", bufs=2)
                        for t in range(2):
                            dinv = sml.tile([128, 1], F32, tag="dinv")
                            nc.vector.reciprocal(dinv[:],
                                                 oe_ps[:, t, DH:DH + 1])
                            nc.vector.tensor_scalar_mul(
                                osc[:, t, :], oe_ps[:, t, 0:DH], dinv[:])
                            nc.tensor.transpose(ot_ps[:, t, :], osc[:, t, :],
                                                ident_b[:])
                        pb = (h % 2) * DH
                        nc.scalar.copy(
                            otr[pb:pb + DH, h // 2, :].rearrange(
                                "p (t n) -> p t n", n=128), ot_ps)
                    for t in range(2):
                        j = jj * 2 + t
                        y_ps = [ctx0_t[:], ctx1_t[:]]
                        for nb in range(2):
                            for ch in range(2):
                                nc.tensor.matmul(
                                    y_ps[nb], otr[:, ch, t * 128:(t + 1) * 128],
                                    wo_b[:, ch * 1024 + nb * 512:
                                         ch * 1024 + nb * 512 + 512],
                                    start=(ch == 0), stop=(ch == 1))
                        y_s = strm.tile([128, 1024], F32, tag="ys", bufs=2)
                        nc.vector.tensor_copy(y_s[:, 0:512], y_ps[0])
                        nc.scalar.copy(y_s[:, 512:1024], y_ps[1])
                        nc.sync.dma_start(y.ap()[j * 128:(j + 1) * 128, :],
                                          y_s[:])

    nc.compile()
    return nc


_prog = None


def _build_in_maps(inputs):
    return _make_in_maps(**inputs)


def _make_in_maps(x, Wq, Wk, Wv, Wo, bo, proj):
    x = np.asarray(x, np.float32)
    projc = np.ascontiguousarray(CNORM * np.asarray(proj, np.float32).T)
    identm = np.eye(128, dtype=np.float32)
    xTb = [np.ascontiguousarray(x[b].T) for b in range(B)]
    in_maps = []
    for c in range(8):
        b, g = c // 4, c % 4
        hs, he = g * 256, g * 256 + 256
        woT = np.asarray(Wo, np.float32)[:, hs:he].T          # [256, 1024]
        woP = np.concatenate([woT[:128], woT[128:]], axis=1)  # [128, 2048]
        in_maps.append({
            "xT": xTb[b],
            "wqT": np.ascontiguousarray(np.asarray(Wq, np.float32)[hs:he].T),
            "wkT": np.ascontiguousarray(np.asarray(Wk, np.float32)[hs:he].T),
            "wvT": np.ascontiguousarray(np.asarray(Wv, np.float32)[hs:he].T),
            "woP": np.ascontiguousarray(woP),
            "projc": projc,
            "ident": identm,
        })
    return in_maps


def kernel(x, Wq, Wk, Wv, Wo, bo, proj):
    global _prog
    if _prog is None:
        _prog = build()
    in_maps = _make_in_maps(x, Wq, Wk, Wv, Wo, bo, proj)
    res = run_bass_kernel_spmd(_prog, in_maps, core_ids=list(range(8)))
    out = np.zeros((B, N, D), np.float32)
    for c in range(8):
        out[c // 4] += res.results[c]["y"]
    out += np.asarray(bo, np.float32)[None, None, :]
    return out
